# revision 24
# baseline (speedup 1.0000x reference)
"""Trainium2 Bass kernel for nn_BertNetModel_76218489634925.

Model: emission matmul -> Gumbel-CRF sampling -> LSTM decoder (H=768,
255 steps) -> output projection to V=30522 vocab with log-softmax loss.

Split of work:
  Host (numpy): emission [8,256,20], CRF forward/backward sampling (serial,
    K=20 -- tiny), entropy term, embedding gather, final scalar combine.
  Device (8 NeuronCores, SPMD):
    phase 1: gx = dec_in @ W_ih.T + b_lstm   (float32r matmuls)
    phase 2: 255-step LSTM recurrence, replicated on all cores
             (W_hh stationary on the PE in bf16, cell state fp32)
    phase 3: output projection, W_out sharded over vocab across the 8
             cores (bf16), fused exp+row-sum -> per-core partial softmax
             denominators (the "all-reduce for the softmax normalizer"
             is the host-side log(sum of per-core partials)).

All shapes hardcoded per the problem spec. Self-contained.
"""

import math
import os
import sys
import types

import ml_dtypes
import numpy as np


def _ensure_axon_hooks():
    """Some images lack antenv.axon_hooks; bass_utils imports it when tracing
    under axon. Provide it (pre-boot, so the axon NTFF hook can register)."""
    try:
        import antenv.axon_hooks  # noqa: F401
        return
    except ImportError:
        pass
    try:
        import antenv
    except ImportError:
        return
    mod = types.ModuleType("antenv.axon_hooks")
    mod._hook = None

    def set_axon_ntff_profile_hook(h):
        mod._hook = h

    def get_axon_ntff_profile_hook():
        return mod._hook

    mod.set_axon_ntff_profile_hook = set_axon_ntff_profile_hook
    mod.get_axon_ntff_profile_hook = get_axon_ntff_profile_hook
    sys.modules["antenv.axon_hooks"] = mod
    antenv.axon_hooks = mod


_ensure_axon_hooks()

B, T, K, H, V = 8, 256, 20, 768, 30522
TAU = 1.0
Z_BETA = 1.0

NT = T - 1            # 255 LSTM steps
NTOK = NT * B         # 2040 tokens
G4 = 4 * H            # 3072 gate rows
NJT = G4 // 128       # 24 gate row-tiles
NDC = H // 128        # 6 contraction chunks
VSH = 3816            # vocab shard per core (8*3816 = 30528 = V+6 pad)
NCORES = 8
GXBLK = 16            # recurrence gx prefetch block (steps)

LAST_RESULTS = None   # BassKernelResults of the most recent device run


# ---------------------------------------------------------------------------
# Host math (numpy, fp32) -- mirrors reference.py ops
# ---------------------------------------------------------------------------

def _logsumexp(x, axis):
    m = np.max(x, axis=axis, keepdims=True)
    return np.squeeze(m, axis) + np.log(np.sum(np.exp(x - m), axis=axis))


def _log_softmax(x, axis=-1):
    m = np.max(x, axis=axis, keepdims=True)
    s = x - m
    return s - np.log(np.sum(np.exp(s), axis=axis, keepdims=True))


def _softmax(x, axis=-1):
    e = np.exp(x - np.max(x, axis=axis, keepdims=True))
    return e / e.sum(axis=axis, keepdims=True)


def _crf_sample(state_matrix, emission, gumbel):
    transition = state_matrix @ state_matrix.T
    em_t = np.swapaxes(emission, 0, 1)                  # [T,B,K]

    alphas = np.empty((T, B, K), np.float32)
    alphas[0] = em_t[0]
    for t in range(1, T):
        x = alphas[t - 1][:, :, None] + transition[None, :, :]
        alphas[t] = _logsumexp(x, axis=1) + em_t[t]

    logits_T = _log_softmax(alphas[-1], axis=-1)
    y_T = logits_T + gumbel[-1]
    z_T = np.argmax(y_T, axis=-1).astype(np.int32)
    relaxed_T = _softmax(y_T / TAU, axis=-1)

    trans_T = transition.T
    zs = np.empty((T - 1, B), np.int32)
    relaxed_s = np.empty((T - 1, B, K), np.float32)
    z_next = z_T
    for t in range(T - 2, -1, -1):
        logits = _log_softmax(alphas[t] + trans_T[z_next], axis=-1)
        y = logits + gumbel[t]
        z = np.argmax(y, axis=-1).astype(np.int32)
        zs[t] = z
        relaxed_s[t] = _softmax(y / TAU, axis=-1)
        z_next = z

    z_ids = np.concatenate([zs, z_T[None]], axis=0)
    relaxed = np.concatenate([relaxed_s, relaxed_T[None]], axis=0)
    return z_ids.T.astype(np.int32), relaxed


def _host_prepare(x_emb, state_matrix, embeddings, gumbel, x, attention_mask):
    x = np.asarray(x)
    mask = np.asarray(attention_mask).astype(np.float32)
    emission = np.einsum('bth,kh->btk', x_emb, state_matrix).astype(np.float32)
    z_sample, relaxed = _crf_sample(state_matrix, emission, gumbel)
    z_sample_emb = np.swapaxes(relaxed, 0, 1) @ state_matrix
    logp_e = _log_softmax(emission, axis=-1)
    ent_tok = -(np.exp(logp_e) * logp_e).sum(-1)
    ent = float((ent_tok * mask).sum() / mask.sum())
    word_emb = embeddings[x]
    dec_in = (z_sample_emb + word_emb)[:, :-1]
    dec_flat = np.ascontiguousarray(np.swapaxes(dec_in, 0, 1)).reshape(NTOK, H)
    return z_sample, ent, dec_flat.astype(np.float32), mask


# ---------------------------------------------------------------------------
# Device kernel
# ---------------------------------------------------------------------------

_NC_CACHE = None


def _build_nc(n_steps=NT):
    import concourse.bass as bass
    import concourse.mybir as mybir
    import concourse.tile as tile
    from concourse import bacc

    dt = mybir.dt
    AF = mybir.ActivationFunctionType
    ntok = n_steps * B

    nc = bacc.Bacc("TRN2", target_bir_lowering=False, debug=False)

    # I/O. Host ships every tensor pre-laid-out exactly as SBUF wants it:
    # partition = (d mod 128) for contraction operands. Gate rows are
    # host-permuted to [i,g,f,o].
    dec_d = nc.dram_tensor("dec_inT", [128, NDC * ntok], dt.float32r,
                           kind="ExternalInput").ap()
    wih_d = nc.dram_tensor("W_ihT", [128, NDC * G4], dt.float32r,
                           kind="ExternalInput").ap()
    whh_d = nc.dram_tensor("W_hhT", [128, NDC * G4], dt.bfloat16,
                           kind="ExternalInput").ap()
    bl_d = nc.dram_tensor("b_lstmP", [128, NJT], dt.float32,
                          kind="ExternalInput").ap()
    wout_d = nc.dram_tensor("W_outT", [128, NDC * VSH], dt.bfloat16,
                            kind="ExternalInput").ap()
    id_d = nc.dram_tensor("ident", [128, 128], dt.float32,
                          kind="ExternalInput").ap()

    n_mt = math.ceil(ntok / 128)
    se_d = nc.dram_tensor("sumexp", [128, n_mt], dt.float32,
                          kind="ExternalOutput").ap()
    hs_d = nc.dram_tensor("hs_out", [128, NDC * ntok], dt.bfloat16,
                          kind="ExternalOutput").ap()

    gx_d = nc.dram_tensor("gx_scratch", [NJT, 128, ntok], dt.float32).ap()

    with tile.TileContext(nc) as tc:
        # ------------------------------------------------------ phase 1: gx
        # dc-outer loop + per-slab DMAs so matmuls start after the first
        # contraction slab lands instead of after the full 16 MB.
        with tc.tile_pool(name="p1", bufs=1) as p1, \
             tc.tile_pool(name="p1s", bufs=4) as p1s, \
             tc.tile_pool(name="p1p", bufs=8, space="PSUM") as p1p:
            wih_s = []
            dec_s = []
            for dc in range(NDC):
                w = p1.tile([128, G4], dt.float32r, tag=f"wih{dc}")
                nc.sync.dma_start(w[:], wih_d[:, dc * G4:(dc + 1) * G4])
                wih_s.append(w)
                d_ = p1.tile([128, ntok], dt.float32r, tag=f"dec{dc}")
                nc.sync.dma_start(d_[:], dec_d[:, dc * ntok:(dc + 1) * ntok])
                dec_s.append(d_)
            bl = p1.tile([128, NJT], dt.float32)
            nc.sync.dma_start(bl[:], bl_d)

            # warm the PE HAM during the input DMA wait (~3.4us of matmul
            # activity flips the clock gate 1.2 -> 2.4 GHz)
            warm = p1.tile([128, 512], dt.float32r)
            nc.vector.memset(warm[:].bitcast(dt.float32), 0.0)
            wps = p1p.tile([128, 512], dt.float32, tag="p1p")
            for _ in range(20):
                nc.tensor.matmul(wps[:], warm[:, :128], warm[:],
                                 start=True, stop=True)

            n_nt = math.ceil(ntok / 512)
            for jt in range(NJT):
                pss = [p1p.tile([128, 512], dt.float32, tag="p1p",
                                name=f"p1ps_{jt}_{i}")
                       for i in range(n_nt)]
                for dc in range(NDC):
                    for nt in range(n_nt):
                        nsz = min(512, ntok - nt * 512)
                        nc.tensor.matmul(
                            pss[nt][:, :nsz],
                            wih_s[dc][:, jt * 128:(jt + 1) * 128],
                            dec_s[dc][:, nt * 512:nt * 512 + nsz],
                            start=(dc == 0), stop=(dc == NDC - 1))
                for nt in range(n_nt):
                    nsz = min(512, ntok - nt * 512)
                    st = p1s.tile([128, 512], dt.float32, tag="p1st")
                    # gates_x + b_lstm (per-partition bias for this row-tile)
                    nc.scalar.activation(st[:, :nsz], pss[nt][:, :nsz],
                                         AF.Identity, bias=bl[:, jt:jt + 1])
                    nc.sync.dma_start(gx_d[jt, :, nt * 512:nt * 512 + nsz],
                                      st[:, :nsz])

        # --------------------------------------------- phase 2: LSTM scan
        with tc.tile_pool(name="big", bufs=1) as big, \
             tc.tile_pool(name="gxb", bufs=2) as gxb, \
             tc.tile_pool(name="sm", bufs=2) as sm:

            whh = big.tile([128, NDC * G4], dt.bfloat16)
            nc.sync.dma_start(whh[:], whh_d)
            ident = big.tile([128, 128], dt.float32)
            nc.sync.dma_start(ident[:], id_d)
            # hs layout: col = dc*ntok + t*B + b  (dc-major so matmul operands
            # slice contiguously; the DVE h-write is the strided one)
            hs = big.tile([128, NDC * ntok], dt.bfloat16)
            hs_v = hs[:].rearrange("p (d tb) -> p d tb", d=NDC)
            hzero = big.tile([128, 48], dt.bfloat16)
            nc.vector.memset(hzero[:], 0.0)
            czero = big.tile([128, 48], dt.float32)
            nc.vector.memset(czero[:], 0.0)

            # projection weights: no deps, DMA overlaps the whole scan
            wout = big.tile([128, NDC * VSH], dt.bfloat16)
            nc.sync.dma_start(wout[:], wout_d)

            gx_v = gx_d.rearrange("j p t -> p j t")

            c_prev = czero
            cur_raw = [None]
            cur_gxs = [None]

            def whh_tile(dc, jt):
                return whh[:, dc * G4 + jt * 128:dc * G4 + (jt + 1) * 128]

            ps2_cm = tc.tile_pool(name="ps2", bufs=2, space="PSUM")
            ps2 = ps2_cm.__enter__()
            psf_cm = tc.tile_pool(name="psf", bufs=1, space="PSUM")
            psf = psf_cm.__enter__()
            psp_cm = tc.tile_pool(name="psp", bufs=1, space="PSUM")
            psp = psp_cm.__enter__()

            # interleaved projection state
            n_vt = math.ceil(VSH / 512)
            ssb = big.tile([128, n_mt], dt.float32)
            nc.vector.memset(ssb[:], 0.0)
            acc_tiles = {}
            ex_done = []

            def emit_proj_unit(u):
                mt, nt = divmod(u, n_vt)
                mtok = min(128, ntok - mt * 128)
                nsz = min(512, VSH - nt * 512)
                if nt == 0:
                    acc_tiles[mt] = sm.tile([128, n_vt], dt.float32,
                                            tag="acc", name=f"acc_{mt}")
                pp = psp.tile([128, 512], dt.float32, tag="proj",
                              name=f"proj_{u}")
                for dc in range(NDC):
                    nc.tensor.matmul(
                        pp[:mtok, :nsz],
                        hs_v[:, dc, mt * 128:mt * 128 + mtok],
                        wout[:, dc * VSH + nt * 512:dc * VSH + nt * 512 + nsz],
                        start=(dc == 0), stop=(dc == NDC - 1))
                ex = sm.tile([128, 512], dt.float32, tag="ex",
                             name=f"ex_{u}")
                nc.scalar.activation(ex[:mtok, :nsz], pp[:mtok, :nsz],
                                     AF.Exp,
                                     accum_out=acc_tiles[mt][:mtok, nt:nt + 1])

            n_units = n_mt * n_vt
            next_u = 0
            for t in range(n_steps):
                blk, tl = divmod(t, GXBLK)
                if tl == 0:
                    s0, s1 = blk * GXBLK, min((blk + 1) * GXBLK, n_steps)
                    nblk = s1 - s0
                    raw = gxb.tile([128, NJT * GXBLK * B], dt.float32,
                                   tag="gxraw")
                    nc.sync.dma_start(
                        raw[:].rearrange("p (j tb) -> p j tb", j=NJT)
                        [:, :, :nblk * B],
                        gx_v[:, :, s0 * B:s1 * B])
                    # reorder jt-major -> step-major so each step reads one
                    # contiguous [128,192] slice (matmul-legal AP)
                    gxs = gxb.tile([128, GXBLK * NJT * B], dt.float32,
                                   tag="gxs")
                    nc.vector.tensor_copy(
                        gxs[:].rearrange("p (t j b) -> p t j b", t=GXBLK,
                                         j=NJT)[:, :nblk],
                        raw[:].rearrange("p (j t b) -> p t j b", j=NJT,
                                         t=GXBLK)[:, :nblk])
                    cur_gxs[0] = gxs
                gxs = cur_gxs[0]
                gx0 = tl * (NJT * B)

                hp = None if t == 0 else (t - 1) * B

                def h_rhs(dc):
                    return (hzero[:, dc * 8:(dc + 1) * 8] if hp is None
                            else hs_v[:, dc, hp:hp + 8])

                # one PSUM bank per gate (order i,g,f,o) so the ACT/DVE chain
                # of earlier gates overlaps later gates' matmuls without
                # PE<->ACT bank serialization.
                gps = []
                for g, tag in enumerate(("pi", "pg", "pf")):
                    pool_ = psf if tag == "pf" else ps2
                    pg_ = pool_.tile([128, 48], dt.float32, tag=tag,
                                     name=f"ps_{tag}_{t}")
                    for j in range(6):
                        jt = g * 6 + j
                        for dc in range(NDC):
                            nc.tensor.matmul(
                                pg_[:, j * 8:(j + 1) * 8],
                                whh_tile(dc, jt), h_rhs(dc),
                                start=(dc == 0), stop=(dc == NDC - 1))
                    gps.append(pg_)
                # o gate: seed PSUM with gx via identity matmul, accumulate
                # W_hh on top -> sigmoid reads PSUM directly (shortest tail)
                ps_o = ps2.tile([128, 48], dt.float32, tag="po")
                nc.tensor.matmul(ps_o[:], ident[:],
                                 gxs[:, gx0 + 144:gx0 + 192],
                                 start=True, stop=False,
                                 skip_group_check=True)
                for j in range(6):
                    jt = 18 + j
                    for dc in range(NDC):
                        nc.tensor.matmul(
                            ps_o[:, j * 8:(j + 1) * 8],
                            whh_tile(dc, jt), h_rhs(dc),
                            start=False,
                            stop=(j == 5 and dc == NDC - 1),
                            skip_group_check=True)

                def gate_sum(g, tag):
                    gs_ = sm.tile([128, 48], dt.float32, tag="g" + tag,
                                  name=f"gs_{tag}_{t}")
                    nc.vector.tensor_add(
                        gs_[:], gps[g][:],
                        gxs[:, gx0 + g * 48:gx0 + (g + 1) * 48])
                    return gs_

                s_i = sm.tile([128, 48], dt.float32, tag="si")
                nc.scalar.activation(s_i[:], gate_sum(0, "i")[:], AF.Sigmoid)
                t_g = sm.tile([128, 48], dt.float32, tag="tg")
                nc.scalar.activation(t_g[:], gate_sum(1, "g")[:], AF.Tanh)
                t1 = sm.tile([128, 48], dt.float32, tag="t1")
                nc.vector.tensor_mul(t1[:], s_i[:], t_g[:])
                s_f = sm.tile([128, 48], dt.float32, tag="sf")
                nc.scalar.activation(s_f[:], gate_sum(2, "f")[:], AF.Sigmoid)

                c_new = sm.tile([128, 48], dt.float32, tag="c")
                nc.vector.tensor_mul(c_new[:], s_f[:], c_prev[:])
                nc.vector.tensor_add(c_new[:], c_new[:], t1[:])
                tanh_c = sm.tile([128, 48], dt.float32, tag="tc")
                nc.scalar.activation(tanh_c[:], c_new[:], AF.Tanh)

                s_o = sm.tile([128, 48], dt.float32, tag="so")
                nc.scalar.activation(s_o[:], ps_o[:], AF.Sigmoid)

                nc.vector.tensor_mul(
                    hs_v[:, :, t * B:(t + 1) * B],
                    s_o[:].rearrange("p (d b) -> p d b", d=NDC),
                    tanh_c[:].rearrange("p (d b) -> p d b", d=NDC))
                c_prev = c_new

                if (t >= 16 and t % 2 == 0 and next_u < n_units
                        and (next_u // n_vt) * 16 + 15 <= t - 1):
                    emit_proj_unit(next_u)
                    next_u += 1

            # drain remaining projection units + the per-m-tile reduces
            while next_u < n_units:
                emit_proj_unit(next_u)
                next_u += 1
            for mt in range(n_mt):
                mtok = min(128, ntok - mt * 128)
                nc.vector.reduce_sum(ssb[:mtok, mt:mt + 1],
                                     acc_tiles[mt][:mtok, :],
                                     axis=mybir.AxisListType.X)

            nc.sync.dma_start(se_d, ssb[:])
            nc.sync.dma_start(hs_d, hs[:])
            psp_cm.__exit__(None, None, None)
            psf_cm.__exit__(None, None, None)
            ps2_cm.__exit__(None, None, None)

    nc.compile()
    return nc


def _get_nc():
    global _NC_CACHE
    if _NC_CACHE is None:
        _NC_CACHE = _build_nc()
    return _NC_CACHE


# gate-row permutation: reference order [i,f,g,o] -> device order [i,g,f,o]
# (lets the c-update chain overlap the f/o matmuls)
_PERM = np.concatenate([np.arange(0, H), np.arange(2 * H, 3 * H),
                        np.arange(H, 2 * H), np.arange(3 * H, 4 * H)])


def _pack_contraction(a_t):
    """[H, X] fp32 -> [128, NDC*X]: out[p, dc*X+x] = a_t[dc*128+p, x]."""
    Hh, X = a_t.shape
    assert Hh == H
    return np.ascontiguousarray(
        a_t.reshape(NDC, 128, X).transpose(1, 0, 2)).reshape(128, NDC * X)


def _make_in_maps(dec_flat, W_ih, W_hh, b_lstm, W_out, n_steps=NT):
    ntok = n_steps * B
    bf16 = ml_dtypes.bfloat16

    W_ih = W_ih[_PERM]
    W_hh = W_hh[_PERM]
    b_lstm = b_lstm[_PERM]
    dec_inT = _pack_contraction(np.ascontiguousarray(dec_flat[:ntok].T))
    W_ihT = _pack_contraction(np.ascontiguousarray(W_ih.T))
    W_hhT = _pack_contraction(np.ascontiguousarray(W_hh.T)).astype(bf16)
    b_p = np.ascontiguousarray(b_lstm.reshape(NJT, 128).T).astype(np.float32)

    Wo_pad = np.zeros((NCORES * VSH, H), np.float32)
    Wo_pad[:V] = W_out
    in_maps = []
    for c in range(NCORES):
        shard = Wo_pad[c * VSH:(c + 1) * VSH]          # [VSH, H]
        woutT = _pack_contraction(np.ascontiguousarray(shard.T)).astype(bf16)
        in_maps.append(dict(dec_inT=dec_inT, W_ihT=W_ihT, W_hhT=W_hhT,
                            b_lstmP=b_p, W_outT=woutT,
                            ident=np.eye(128, dtype=np.float32)))
    return in_maps


def _run_device(dec_flat, W_ih, W_hh, b_lstm, W_out, n_steps=NT):
    global LAST_RESULTS
    from concourse import bass_utils

    ntok = n_steps * B
    in_maps = _make_in_maps(dec_flat, W_ih, W_hh, b_lstm, W_out, n_steps)
    nc = _get_nc() if n_steps == NT else _build_nc(n_steps)
    res = bass_utils.run_bass_kernel_spmd(nc, in_maps, list(range(NCORES)))
    LAST_RESULTS = res

    n_mt = math.ceil(ntok / 128)
    sumexp = np.zeros(ntok, np.float64)
    for c in range(NCORES):
        s = np.asarray(res.results[c]["sumexp"]).T.reshape(n_mt * 128)[:ntok]
        s = s.astype(np.float64)
        if c == NCORES - 1:
            s = s - (NCORES * VSH - V)   # padded vocab rows contribute exp(0)
        sumexp += s
    hs_b = np.asarray(res.results[0]["hs_out"])        # [128, NDC*ntok] bf16
    hs_flat = (hs_b.astype(np.float32)
               .reshape(128, NDC, ntok)
               .transpose(2, 1, 0)                     # [token, dc, p]
               .reshape(ntok, H))
    return hs_flat, sumexp.astype(np.float32)


# ---------------------------------------------------------------------------
# Entry point
# ---------------------------------------------------------------------------

def kernel(x_emb, state_matrix, embeddings, W_ih, W_hh, b_lstm, W_out, b_out,
           gumbel, x, attention_mask):
    x_emb = np.asarray(x_emb, np.float32)
    state_matrix = np.asarray(state_matrix, np.float32)
    embeddings = np.asarray(embeddings, np.float32)
    W_ih = np.asarray(W_ih, np.float32)
    W_hh = np.asarray(W_hh, np.float32)
    b_lstm = np.asarray(b_lstm, np.float32)
    W_out = np.asarray(W_out, np.float32)
    b_out = np.asarray(b_out, np.float32)
    gumbel = np.asarray(gumbel, np.float32)

    z_sample, ent, dec_flat, mask = _host_prepare(
        x_emb, state_matrix, embeddings, gumbel, x, attention_mask)

    hs_flat, sumexp = _run_device(dec_flat, W_ih, W_hh, b_lstm, W_out)

    x_np = np.asarray(x)
    tgt = np.swapaxes(x_np[:, 1:], 0, 1).reshape(-1)
    W_tgt = W_out[tgt]
    logit_tgt = np.einsum('td,td->t', hs_flat, W_tgt) + b_out[tgt]
    tok_lp = logit_tgt - np.log(sumexp)
    tgt_mask = np.swapaxes(mask[:, 1:], 0, 1).reshape(-1)
    p_log_prob = (tok_lp * tgt_mask).sum() / tgt_mask.sum()

    loss = -(p_log_prob + Z_BETA * ent)
    return np.float32(loss), z_sample


# revision 27
# speedup vs baseline: 1.0945x; 1.0945x over previous
"""Trainium2 Bass kernel for nn_BertNetModel_76218489634925.

Model: emission matmul -> Gumbel-CRF sampling -> LSTM decoder (H=768,
255 steps) -> output projection to V=30522 vocab with log-softmax loss.

Split of work:
  Host (numpy): emission [8,256,20], CRF forward/backward sampling (serial,
    K=20 -- tiny), entropy term, embedding gather, final scalar combine.
  Device (8 NeuronCores, SPMD):
    phase 1: gx = dec_in @ W_ih.T + b_lstm   (float32r matmuls)
    phase 2: 255-step LSTM recurrence, replicated on all cores
             (W_hh stationary on the PE in bf16, cell state fp32)
    phase 3: output projection, W_out sharded over vocab across the 8
             cores (bf16), fused exp+row-sum -> per-core partial softmax
             denominators (the "all-reduce for the softmax normalizer"
             is the host-side log(sum of per-core partials)).

All shapes hardcoded per the problem spec. Self-contained.
"""

import math
import os
import sys
import types

import ml_dtypes
import numpy as np


def _ensure_axon_hooks():
    """Some images lack antenv.axon_hooks; bass_utils imports it when tracing
    under axon. Provide it (pre-boot, so the axon NTFF hook can register)."""
    try:
        import antenv.axon_hooks  # noqa: F401
        return
    except ImportError:
        pass
    try:
        import antenv
    except ImportError:
        return
    mod = types.ModuleType("antenv.axon_hooks")
    mod._hook = None

    def set_axon_ntff_profile_hook(h):
        mod._hook = h

    def get_axon_ntff_profile_hook():
        return mod._hook

    mod.set_axon_ntff_profile_hook = set_axon_ntff_profile_hook
    mod.get_axon_ntff_profile_hook = get_axon_ntff_profile_hook
    sys.modules["antenv.axon_hooks"] = mod
    antenv.axon_hooks = mod


_ensure_axon_hooks()

B, T, K, H, V = 8, 256, 20, 768, 30522
TAU = 1.0
Z_BETA = 1.0

NT = T - 1            # 255 LSTM steps
NTOK = NT * B         # 2040 tokens
G4 = 4 * H            # 3072 gate rows
NJT = G4 // 128       # 24 gate row-tiles
NDC = H // 128        # 6 contraction chunks
VSH = 3816            # vocab shard per core (8*3816 = 30528 = V+6 pad)
NCORES = 8
GXBLK = 16            # recurrence gx prefetch block (steps)

LAST_RESULTS = None   # BassKernelResults of the most recent device run


# ---------------------------------------------------------------------------
# Host math (numpy, fp32) -- mirrors reference.py ops
# ---------------------------------------------------------------------------

def _logsumexp(x, axis):
    m = np.max(x, axis=axis, keepdims=True)
    return np.squeeze(m, axis) + np.log(np.sum(np.exp(x - m), axis=axis))


def _log_softmax(x, axis=-1):
    m = np.max(x, axis=axis, keepdims=True)
    s = x - m
    return s - np.log(np.sum(np.exp(s), axis=axis, keepdims=True))


def _softmax(x, axis=-1):
    e = np.exp(x - np.max(x, axis=axis, keepdims=True))
    return e / e.sum(axis=axis, keepdims=True)


def _crf_sample(state_matrix, emission, gumbel):
    transition = state_matrix @ state_matrix.T
    em_t = np.swapaxes(emission, 0, 1)                  # [T,B,K]

    alphas = np.empty((T, B, K), np.float32)
    alphas[0] = em_t[0]
    for t in range(1, T):
        x = alphas[t - 1][:, :, None] + transition[None, :, :]
        alphas[t] = _logsumexp(x, axis=1) + em_t[t]

    logits_T = _log_softmax(alphas[-1], axis=-1)
    y_T = logits_T + gumbel[-1]
    z_T = np.argmax(y_T, axis=-1).astype(np.int32)
    relaxed_T = _softmax(y_T / TAU, axis=-1)

    trans_T = transition.T
    zs = np.empty((T - 1, B), np.int32)
    relaxed_s = np.empty((T - 1, B, K), np.float32)
    z_next = z_T
    for t in range(T - 2, -1, -1):
        logits = _log_softmax(alphas[t] + trans_T[z_next], axis=-1)
        y = logits + gumbel[t]
        z = np.argmax(y, axis=-1).astype(np.int32)
        zs[t] = z
        relaxed_s[t] = _softmax(y / TAU, axis=-1)
        z_next = z

    z_ids = np.concatenate([zs, z_T[None]], axis=0)
    relaxed = np.concatenate([relaxed_s, relaxed_T[None]], axis=0)
    return z_ids.T.astype(np.int32), relaxed


def _host_prepare(x_emb, state_matrix, embeddings, gumbel, x, attention_mask):
    x = np.asarray(x)
    mask = np.asarray(attention_mask).astype(np.float32)
    emission = np.einsum('bth,kh->btk', x_emb, state_matrix).astype(np.float32)
    z_sample, relaxed = _crf_sample(state_matrix, emission, gumbel)
    z_sample_emb = np.swapaxes(relaxed, 0, 1) @ state_matrix
    logp_e = _log_softmax(emission, axis=-1)
    ent_tok = -(np.exp(logp_e) * logp_e).sum(-1)
    ent = float((ent_tok * mask).sum() / mask.sum())
    word_emb = embeddings[x]
    dec_in = (z_sample_emb + word_emb)[:, :-1]
    dec_flat = np.ascontiguousarray(np.swapaxes(dec_in, 0, 1)).reshape(NTOK, H)
    return z_sample, ent, dec_flat.astype(np.float32), mask


# ---------------------------------------------------------------------------
# Device kernel
# ---------------------------------------------------------------------------

_NC_CACHE = None


def _build_nc(n_steps=NT):
    import concourse.bass as bass
    import concourse.mybir as mybir
    import concourse.tile as tile
    from concourse import bacc

    dt = mybir.dt
    AF = mybir.ActivationFunctionType
    ntok = n_steps * B

    nc = bacc.Bacc("TRN2", target_bir_lowering=False, debug=False)

    # I/O. Host ships every tensor pre-laid-out exactly as SBUF wants it:
    # partition = (d mod 128) for contraction operands. Gate rows are
    # host-permuted to [i,g,f,o].
    dec_d = nc.dram_tensor("dec_inT", [128, NDC * ntok], dt.float32r,
                           kind="ExternalInput").ap()
    wih_d = nc.dram_tensor("W_ihT", [128, NDC * G4], dt.float32r,
                           kind="ExternalInput").ap()
    whh_d = nc.dram_tensor("W_hhT", [128, NDC * G4], dt.bfloat16,
                           kind="ExternalInput").ap()
    bl_d = nc.dram_tensor("b_lstmP", [128, NJT], dt.float32,
                          kind="ExternalInput").ap()
    wout_d = nc.dram_tensor("W_outT", [128, NDC * VSH], dt.bfloat16,
                            kind="ExternalInput").ap()
    id_d = nc.dram_tensor("ident", [128, 128], dt.float32,
                          kind="ExternalInput").ap()

    n_mt = math.ceil(ntok / 128)
    se_d = nc.dram_tensor("sumexp", [128, n_mt], dt.float32,
                          kind="ExternalOutput").ap()
    hs_d = nc.dram_tensor("hs_out", [128, NDC * ntok], dt.bfloat16,
                          kind="ExternalOutput").ap()

    gx_d = nc.dram_tensor("gx_scratch", [NJT, 128, ntok], dt.float32).ap()

    with tile.TileContext(nc) as tc:
        # ------------------------------------------------------ phase 1: gx
        # dc-outer loop + per-slab DMAs so matmuls start after the first
        # contraction slab lands instead of after the full 16 MB.
        with tc.tile_pool(name="p1", bufs=1) as p1, \
             tc.tile_pool(name="p1s", bufs=4) as p1s, \
             tc.tile_pool(name="p1p", bufs=8, space="PSUM") as p1p:
            wih_s = []
            dec_s = []
            for dc in range(NDC):
                w = p1.tile([128, G4], dt.float32r, tag=f"wih{dc}")
                nc.sync.dma_start(w[:], wih_d[:, dc * G4:(dc + 1) * G4])
                wih_s.append(w)
                d_ = p1.tile([128, ntok], dt.float32r, tag=f"dec{dc}")
                nc.sync.dma_start(d_[:], dec_d[:, dc * ntok:(dc + 1) * ntok])
                dec_s.append(d_)
            bl = p1.tile([128, NJT], dt.float32)
            nc.sync.dma_start(bl[:], bl_d)

            # warm the PE HAM during the input DMA wait (~3.4us of matmul
            # activity flips the clock gate 1.2 -> 2.4 GHz)
            warm = p1.tile([128, 512], dt.float32r)
            nc.vector.memset(warm[:].bitcast(dt.float32), 0.0)
            wps = p1p.tile([128, 512], dt.float32, tag="p1p")
            for _ in range(48):
                nc.tensor.matmul(wps[:], warm[:, :128], warm[:],
                                 start=True, stop=True)

            n_nt = math.ceil(ntok / 512)
            for jt in range(NJT):
                pss = [p1p.tile([128, 512], dt.float32, tag="p1p",
                                name=f"p1ps_{jt}_{i}")
                       for i in range(n_nt)]
                for dc in range(NDC):
                    for nt in range(n_nt):
                        nsz = min(512, ntok - nt * 512)
                        nc.tensor.matmul(
                            pss[nt][:, :nsz],
                            wih_s[dc][:, jt * 128:(jt + 1) * 128],
                            dec_s[dc][:, nt * 512:nt * 512 + nsz],
                            start=(dc == 0), stop=(dc == NDC - 1))
                for nt in range(n_nt):
                    nsz = min(512, ntok - nt * 512)
                    st = p1s.tile([128, 512], dt.float32, tag="p1st")
                    # gates_x + b_lstm (per-partition bias for this row-tile)
                    nc.scalar.activation(st[:, :nsz], pss[nt][:, :nsz],
                                         AF.Identity, bias=bl[:, jt:jt + 1])
                    nc.sync.dma_start(gx_d[jt, :, nt * 512:nt * 512 + nsz],
                                      st[:, :nsz])

        # --------------------------------------------- phase 2: LSTM scan
        with tc.tile_pool(name="big", bufs=1) as big, \
             tc.tile_pool(name="gxb", bufs=2) as gxb, \
             tc.tile_pool(name="sm", bufs=2) as sm:

            whh = big.tile([128, NDC * G4], dt.bfloat16)
            nc.sync.dma_start(whh[:], whh_d)
            ident = big.tile([128, 128], dt.float32)
            nc.sync.dma_start(ident[:], id_d)
            # hs layout: col = dc*ntok + t*B + b  (dc-major so matmul operands
            # slice contiguously; the DVE h-write is the strided one)
            hs = big.tile([128, NDC * ntok], dt.bfloat16)
            hs_v = hs[:].rearrange("p (d tb) -> p d tb", d=NDC)
            hzero = big.tile([128, 48], dt.bfloat16)
            nc.vector.memset(hzero[:], 0.0)
            czero = big.tile([128, 48], dt.float32)
            nc.vector.memset(czero[:], 0.0)

            # projection weights: no deps, DMA overlaps the whole scan
            wout = big.tile([128, NDC * VSH], dt.bfloat16)
            nc.sync.dma_start(wout[:], wout_d)

            gx_v = gx_d.rearrange("j p t -> p j t")

            c_prev = czero
            cur_raw = [None]
            cur_gxs = [None]

            def whh_tile(dc, jt):
                return whh[:, dc * G4 + jt * 128:dc * G4 + (jt + 1) * 128]

            ps2_cm = tc.tile_pool(name="ps2", bufs=2, space="PSUM")
            ps2 = ps2_cm.__enter__()
            for t in range(n_steps):
                blk, tl = divmod(t, GXBLK)
                if tl == 0:
                    s0, s1 = blk * GXBLK, min((blk + 1) * GXBLK, n_steps)
                    nblk = s1 - s0
                    raw = gxb.tile([128, NJT * GXBLK * B], dt.float32,
                                   tag="gxraw")
                    nc.sync.dma_start(
                        raw[:].rearrange("p (j tb) -> p j tb", j=NJT)
                        [:, :, :nblk * B],
                        gx_v[:, :, s0 * B:s1 * B])
                    # reorder jt-major -> step-major so each step reads one
                    # contiguous [128,192] slice (matmul-legal AP)
                    gxs = gxb.tile([128, GXBLK * NJT * B], dt.float32,
                                   tag="gxs")
                    nc.vector.tensor_copy(
                        gxs[:].rearrange("p (t j b) -> p t j b", t=GXBLK,
                                         j=NJT)[:, :nblk],
                        raw[:].rearrange("p (j t b) -> p t j b", j=NJT,
                                         t=GXBLK)[:, :nblk])
                    cur_gxs[0] = gxs
                gxs = cur_gxs[0]
                gx0 = tl * (NJT * B)

                hp = None if t == 0 else (t - 1) * B

                def h_rhs(dc):
                    return (hzero[:, dc * 8:(dc + 1) * 8] if hp is None
                            else hs_v[:, dc, hp:hp + 8])

                # o-gate PSUM is seeded with gx via an identity matmul
                # emitted FIRST: it has no h dependency, so it executes
                # during the previous step's tail while the PE is idle.
                ps_o = ps2.tile([128, 48], dt.float32, tag="po")
                nc.tensor.matmul(ps_o[:], ident[:],
                                 gxs[:, gx0 + 144:gx0 + 192],
                                 start=True, stop=False,
                                 skip_group_check=True)
                # one PSUM bank per gate (order i,g,f,o) so the ACT/DVE chain
                # of earlier gates overlaps later gates' matmuls without
                # PE<->ACT bank serialization. The i group runs dc-outer:
                # its first matmuls need only the first half of h.
                gps = []
                ps_i = ps2.tile([128, 48], dt.float32, tag="pi",
                                name=f"ps_pi_{t}")
                for dc in range(NDC):
                    for j in range(6):
                        nc.tensor.matmul(
                            ps_i[:, j * 8:(j + 1) * 8],
                            whh_tile(dc, j), h_rhs(dc),
                            start=(dc == 0 and j == 0),
                            stop=(dc == NDC - 1 and j == 5),
                            skip_group_check=True)
                gps.append(ps_i)
                for g, tag in ((1, "pg"), (2, "pf")):
                    pg_ = ps2.tile([128, 48], dt.float32, tag=tag,
                                   name=f"ps_{tag}_{t}")
                    for j in range(6):
                        jt = g * 6 + j
                        for dc in range(NDC):
                            nc.tensor.matmul(
                                pg_[:, j * 8:(j + 1) * 8],
                                whh_tile(dc, jt), h_rhs(dc),
                                start=(dc == 0), stop=(dc == NDC - 1))
                    gps.append(pg_)
                for j in range(6):
                    jt = 18 + j
                    for dc in range(NDC):
                        nc.tensor.matmul(
                            ps_o[:, j * 8:(j + 1) * 8],
                            whh_tile(dc, jt), h_rhs(dc),
                            start=False,
                            stop=(j == 5 and dc == NDC - 1),
                            skip_group_check=True)

                def gate_sum(g, tag):
                    gs_ = sm.tile([128, 48], dt.float32, tag="g" + tag,
                                  name=f"gs_{tag}_{t}")
                    nc.vector.tensor_add(
                        gs_[:], gps[g][:],
                        gxs[:, gx0 + g * 48:gx0 + (g + 1) * 48])
                    return gs_

                s_i = sm.tile([128, 48], dt.float32, tag="si")
                nc.scalar.activation(s_i[:], gate_sum(0, "i")[:], AF.Sigmoid)
                t_g = sm.tile([128, 48], dt.float32, tag="tg")
                nc.scalar.activation(t_g[:], gate_sum(1, "g")[:], AF.Tanh)
                t1 = sm.tile([128, 48], dt.float32, tag="t1")
                nc.vector.tensor_mul(t1[:], s_i[:], t_g[:])
                s_f = sm.tile([128, 48], dt.float32, tag="sf")
                nc.scalar.activation(s_f[:], gate_sum(2, "f")[:], AF.Sigmoid)

                c_new = sm.tile([128, 48], dt.float32, tag="c")
                nc.vector.tensor_mul(c_new[:], s_f[:], c_prev[:])
                nc.vector.tensor_add(c_new[:], c_new[:], t1[:])
                tanh_c = sm.tile([128, 48], dt.float32, tag="tc")
                nc.scalar.activation(tanh_c[:], c_new[:], AF.Tanh)

                s_o = sm.tile([128, 48], dt.float32, tag="so")
                nc.scalar.activation(s_o[:], ps_o[:], AF.Sigmoid)

                nc.vector.tensor_mul(
                    hs_v[:, 0:3, t * B:(t + 1) * B],
                    s_o[:, 0:24].rearrange("p (d b) -> p d b", d=3),
                    tanh_c[:, 0:24].rearrange("p (d b) -> p d b", d=3))
                nc.vector.tensor_mul(
                    hs_v[:, 3:6, t * B:(t + 1) * B],
                    s_o[:, 24:48].rearrange("p (d b) -> p d b", d=3),
                    tanh_c[:, 24:48].rearrange("p (d b) -> p d b", d=3))
                c_prev = c_new

            ps2_cm.__exit__(None, None, None)

            # ------------------------------------- phase 3: projection
            ps3_cm = tc.tile_pool(name="ps3", bufs=8, space="PSUM")
            ps3 = ps3_cm.__enter__()
            ssb = big.tile([128, n_mt], dt.float32)
            nc.vector.memset(ssb[:], 0.0)
            n_vt = math.ceil(VSH / 512)
            for mt in range(n_mt):
                mtok = min(128, ntok - mt * 128)
                acc = sm.tile([128, n_vt], dt.float32, tag="acc")
                for nt in range(n_vt):
                    nsz = min(512, VSH - nt * 512)
                    pp = ps3.tile([128, 512], dt.float32, tag="proj")
                    for dc in range(NDC):
                        nc.tensor.matmul(
                            pp[:mtok, :nsz],
                            hs_v[:, dc, mt * 128:mt * 128 + mtok],
                            wout[:, dc * VSH + nt * 512:dc * VSH + nt * 512 + nsz],
                            start=(dc == 0), stop=(dc == NDC - 1))
                    ex = sm.tile([128, 512], dt.float32, tag="ex")
                    nc.scalar.activation(ex[:mtok, :nsz], pp[:mtok, :nsz],
                                         AF.Exp, accum_out=acc[:mtok, nt:nt + 1])
                nc.vector.reduce_sum(ssb[:mtok, mt:mt + 1], acc[:mtok, :],
                                     axis=mybir.AxisListType.X)

            nc.sync.dma_start(se_d, ssb[:])
            nc.sync.dma_start(hs_d, hs[:])
            ps3_cm.__exit__(None, None, None)

    nc.compile()
    return nc


def _get_nc():
    global _NC_CACHE
    if _NC_CACHE is None:
        _NC_CACHE = _build_nc()
    return _NC_CACHE


# gate-row permutation: reference order [i,f,g,o] -> device order [i,g,f,o]
# (lets the c-update chain overlap the f/o matmuls)
_PERM = np.concatenate([np.arange(0, H), np.arange(2 * H, 3 * H),
                        np.arange(H, 2 * H), np.arange(3 * H, 4 * H)])


def _pack_contraction(a_t):
    """[H, X] fp32 -> [128, NDC*X]: out[p, dc*X+x] = a_t[dc*128+p, x]."""
    Hh, X = a_t.shape
    assert Hh == H
    return np.ascontiguousarray(
        a_t.reshape(NDC, 128, X).transpose(1, 0, 2)).reshape(128, NDC * X)


def _make_in_maps(dec_flat, W_ih, W_hh, b_lstm, W_out, n_steps=NT):
    ntok = n_steps * B
    bf16 = ml_dtypes.bfloat16

    W_ih = W_ih[_PERM]
    W_hh = W_hh[_PERM]
    b_lstm = b_lstm[_PERM]
    dec_inT = _pack_contraction(np.ascontiguousarray(dec_flat[:ntok].T))
    W_ihT = _pack_contraction(np.ascontiguousarray(W_ih.T))
    W_hhT = _pack_contraction(np.ascontiguousarray(W_hh.T)).astype(bf16)
    b_p = np.ascontiguousarray(b_lstm.reshape(NJT, 128).T).astype(np.float32)

    Wo_pad = np.zeros((NCORES * VSH, H), np.float32)
    Wo_pad[:V] = W_out
    in_maps = []
    for c in range(NCORES):
        shard = Wo_pad[c * VSH:(c + 1) * VSH]          # [VSH, H]
        woutT = _pack_contraction(np.ascontiguousarray(shard.T)).astype(bf16)
        in_maps.append(dict(dec_inT=dec_inT, W_ihT=W_ihT, W_hhT=W_hhT,
                            b_lstmP=b_p, W_outT=woutT,
                            ident=np.eye(128, dtype=np.float32)))
    return in_maps


def _run_device(dec_flat, W_ih, W_hh, b_lstm, W_out, n_steps=NT):
    global LAST_RESULTS
    from concourse import bass_utils

    ntok = n_steps * B
    in_maps = _make_in_maps(dec_flat, W_ih, W_hh, b_lstm, W_out, n_steps)
    nc = _get_nc() if n_steps == NT else _build_nc(n_steps)
    res = bass_utils.run_bass_kernel_spmd(nc, in_maps, list(range(NCORES)))
    LAST_RESULTS = res

    n_mt = math.ceil(ntok / 128)
    sumexp = np.zeros(ntok, np.float64)
    for c in range(NCORES):
        s = np.asarray(res.results[c]["sumexp"]).T.reshape(n_mt * 128)[:ntok]
        s = s.astype(np.float64)
        if c == NCORES - 1:
            s = s - (NCORES * VSH - V)   # padded vocab rows contribute exp(0)
        sumexp += s
    hs_b = np.asarray(res.results[0]["hs_out"])        # [128, NDC*ntok] bf16
    hs_flat = (hs_b.astype(np.float32)
               .reshape(128, NDC, ntok)
               .transpose(2, 1, 0)                     # [token, dc, p]
               .reshape(ntok, H))
    return hs_flat, sumexp.astype(np.float32)


# ---------------------------------------------------------------------------
# Entry point
# ---------------------------------------------------------------------------

def kernel(x_emb, state_matrix, embeddings, W_ih, W_hh, b_lstm, W_out, b_out,
           gumbel, x, attention_mask):
    x_emb = np.asarray(x_emb, np.float32)
    state_matrix = np.asarray(state_matrix, np.float32)
    embeddings = np.asarray(embeddings, np.float32)
    W_ih = np.asarray(W_ih, np.float32)
    W_hh = np.asarray(W_hh, np.float32)
    b_lstm = np.asarray(b_lstm, np.float32)
    W_out = np.asarray(W_out, np.float32)
    b_out = np.asarray(b_out, np.float32)
    gumbel = np.asarray(gumbel, np.float32)

    z_sample, ent, dec_flat, mask = _host_prepare(
        x_emb, state_matrix, embeddings, gumbel, x, attention_mask)

    hs_flat, sumexp = _run_device(dec_flat, W_ih, W_hh, b_lstm, W_out)

    x_np = np.asarray(x)
    tgt = np.swapaxes(x_np[:, 1:], 0, 1).reshape(-1)
    W_tgt = W_out[tgt]
    logit_tgt = np.einsum('td,td->t', hs_flat, W_tgt) + b_out[tgt]
    tok_lp = logit_tgt - np.log(sumexp)
    tgt_mask = np.swapaxes(mask[:, 1:], 0, 1).reshape(-1)
    p_log_prob = (tok_lp * tgt_mask).sum() / tgt_mask.sum()

    loss = -(p_log_prob + Z_BETA * ent)
    return np.float32(loss), z_sample
